# revision 2
# baseline (speedup 1.0000x reference)
"""BERT embedding lookup on 8 TRN2 NeuronCores.

Strategy: data-parallel over batch. Core c handles batch rows [4c, 4c+4)
(2048 tokens). Word rows are fetched with dma_gather (512 rows per
instruction; row j lands on partition j%128, slot j//128). Position +
token-type are folded in with one scalar_tensor_tensor per 128-token tile:
    out = word[id] + (posA[s] + tt * diff)
where posA = position_embedding + type_row0 and diff = type_row1 - type_row0
are precomputed on the host (cheap O(S*H) work). No collectives.
"""

import numpy as np

P = 128
H = 768
VOCAB = 30522
SEQ = 512
BATCH = 32
N_CORES = 8
TOK_PER_CORE = BATCH * SEQ // N_CORES  # 2048
T_TILES = TOK_PER_CORE // P  # 16
S_BLOCKS = SEQ // P  # 4
CHUNK = 512  # rows per dma_gather = one local batch row
N_CHUNKS = TOK_PER_CORE // CHUNK  # 4
TPC = CHUNK // P  # tiles per chunk = 4

_CACHE = {}


def _build(gather_bufs=3, res_bufs=3):
    from concourse import bacc, mybir
    import concourse.tile as tile

    nc = bacc.Bacc(
        "TRN2", target_bir_lowering=False, debug=False, num_devices=N_CORES
    )
    f32 = mybir.dt.float32
    i16 = mybir.dt.int16

    wemb = nc.dram_tensor("wemb", [VOCAB, H], f32, kind="ExternalInput").ap()
    posA = nc.dram_tensor("posA", [SEQ, H], f32, kind="ExternalInput").ap()
    diffr = nc.dram_tensor("diffr", [P, H], f32, kind="ExternalInput").ap()
    idsw = nc.dram_tensor(
        "idsw", [P, TOK_PER_CORE // 16], i16, kind="ExternalInput"
    ).ap()
    ttf = nc.dram_tensor("ttf", [P, T_TILES], f32, kind="ExternalInput").ap()
    out = nc.dram_tensor("out", [TOK_PER_CORE, H], f32, kind="ExternalOutput").ap()

    with tile.TileContext(nc) as tc:
        with (
            tc.tile_pool(name="consts", bufs=1) as consts,
            tc.tile_pool(name="gather", bufs=gather_bufs) as gpool,
            tc.tile_pool(name="res", bufs=res_bufs) as rpool,
        ):
            idsw_sb = consts.tile([P, TOK_PER_CORE // 16], i16)
            nc.sync.dma_start(out=idsw_sb[:], in_=idsw[:])
            ttf_sb = consts.tile([P, T_TILES], f32)
            nc.sync.dma_start(out=ttf_sb[:], in_=ttf[:])
            diff_sb = consts.tile([P, H], f32)
            nc.sync.dma_start(out=diff_sb[:], in_=diffr[:])
            pos_sb = []
            for sb in range(S_BLOCKS):
                pt = consts.tile([P, H], f32, tag=f"pos{sb}")
                nc.sync.dma_start(out=pt[:], in_=posA[sb * P : (sb + 1) * P, :])
                pos_sb.append(pt)

            nreg = nc.gpsimd.to_reg(CHUNK)

            for g in range(N_CHUNKS):
                wt = gpool.tile([P, TPC * H], f32)
                nc.gpsimd.dma_gather(
                    out_ap=wt[:].rearrange("p (t h) -> p t h", h=H),
                    in_ap=wemb[:],
                    idxs_ap=idsw_sb[:, g * (CHUNK // 16) : (g + 1) * (CHUNK // 16)],
                    num_idxs=CHUNK,
                    num_idxs_reg=nreg,
                    elem_size=H,
                )
                res = rpool.tile([P, TPC * H], f32)
                for tl in range(TPC):
                    t = g * TPC + tl
                    seg = slice(tl * H, (tl + 1) * H)
                    # res = diff * tt + posA[s-block]; s-block == tl here
                    nc.vector.scalar_tensor_tensor(
                        out=res[:, seg],
                        in0=diff_sb[:],
                        scalar=ttf_sb[:, t : t + 1],
                        in1=pos_sb[tl][:],
                        op0=mybir.AluOpType.mult,
                        op1=mybir.AluOpType.add,
                    )
                    nc.vector.tensor_add(
                        out=res[:, seg], in0=res[:, seg], in1=wt[:, seg]
                    )
                nc.sync.dma_start(
                    out=out[g * CHUNK : (g + 1) * CHUNK, :].rearrange(
                        "(t p) h -> p t h", p=P
                    ),
                    in_=res[:].rearrange("p (t h) -> p t h", h=H),
                )

    nc.compile()
    return nc


def _get_nc():
    if "nc" not in _CACHE:
        _CACHE["nc"] = _build()
    return _CACHE["nc"]


def _prep_inputs(
    input_ids, token_type_ids, word_embedding, position_embedding, token_type_embedding
):
    ids = np.asarray(input_ids, dtype=np.int64).reshape(N_CORES, TOK_PER_CORE)
    # wrapped int16 layout: token j -> [j % 16, j // 16], replicated to 128 rows
    w = ids.reshape(N_CORES, TOK_PER_CORE // 16, 16).transpose(0, 2, 1)  # [N,16,128]
    idsw = np.ascontiguousarray(np.tile(w, (1, P // 16, 1)).astype(np.int16))
    ttf = np.ascontiguousarray(
        np.asarray(token_type_ids, dtype=np.float32)
        .reshape(N_CORES, T_TILES, P)
        .transpose(0, 2, 1)
    )
    wemb = np.ascontiguousarray(np.asarray(word_embedding, dtype=np.float32))
    pos = np.asarray(position_embedding, dtype=np.float32)
    typ = np.asarray(token_type_embedding, dtype=np.float32)
    posA = np.ascontiguousarray(pos + typ[0][None, :])
    diffr = np.ascontiguousarray(np.broadcast_to(typ[1] - typ[0], (P, H)))
    return [
        {"wemb": wemb, "posA": posA, "diffr": diffr, "idsw": idsw[c], "ttf": ttf[c]}
        for c in range(N_CORES)
    ]


def kernel(
    input_ids, token_type_ids, word_embedding, position_embedding, token_type_embedding
):
    from concourse.bass_utils import run_bass_kernel_spmd

    nc = _get_nc()
    in_maps = _prep_inputs(
        input_ids,
        token_type_ids,
        word_embedding,
        position_embedding,
        token_type_embedding,
    )
    r = run_bass_kernel_spmd(nc, in_maps, core_ids=list(range(N_CORES)))
    out = np.stack([r.results[c]["out"] for c in range(N_CORES)], axis=0)
    return out.reshape(BATCH, SEQ, H)
